# revision 43
# baseline (speedup 1.0000x reference)
"""Bahdanau-style additive attention on 8 TRN2 NeuronCores.

  hidden = tanh(q @ Wq + k @ Wk)        (B, L, H)
  scores = hidden @ v_param             (B, L)
  attn   = softmax(scores, axis=-1)
  out    = attn @ v                     (B, D)

Sharding: data-parallel over batch — 4 batches per core (B=32, 8 cores).

Per-core device pipeline:

  W1  preT[H, L]   = Wk.T @ kT          stationary=Wk, moving=host-transposed k
  ACT hiddenT      = tanh(preT + qWq_b) per-partition bias
  W2  scores[L, 1] = hiddenT.T @ vp     stationary=hidden chunk -> score COLUMNS
  ACT w = exp(scores)                   no max-subtraction (|scores| << 88)
  W3  acc[1, D+1]  = w.T @ [v | 1]      stationary=w column (float32r), the
                                        ones column gives the softmax
                                        denominator for free
  host: out = acc[:D] / acc[D]

MODE="bf16" (default): k, hidden, w, v all bf16 — halves DMA bytes vs
hilo/f32r (the kernel is DMA-bound) and runs W1/W2 as single matmuls;
rel err ~9e-3 against the f32 reference (gate is 2e-2). W1 runs as 4
back-to-back matmuls per 2048-col k tile (one wk stationary load, psum
banks pre0-3); W2 stops alternate two psum banks so readouts overlap;
exp runs per group as two [128,8] ACTs whose outputs land in
even/odd-interleaved w columns (v is host-permuted to match). v is
fully prefetched into SBUF upfront on the gpsimd SWDGE queue; k
streams on the sync HWDGE queue with 3-tile lookahead.

MODE="hilo": W1/W2 run as bf16 hi+lo split pairs (x = hi + lo exactly to
~2^-17), 3 matmuls each with the negligible lo*lo term dropped — fp32-grade
precision at bf16 PE speed, same DMA bytes as fp32.
MODE="f32r": W1/W2 in float32r (TF32-like, ~11-bit-mantissa RNE) — fewer
PE instructions, ~1e-3 relative error.
"""

import ml_dtypes
import numpy as np

import concourse.bass as bass
import concourse.mybir as mybir
from concourse.tile import TileContext

B, L, D, H = 32, 8192, 128, 128
NCORES = 8
BPC = B // NCORES  # batches per core
CHUNK = 512  # L positions per W1/tanh chunk (psum bank limit)
NCH = L // CHUNK  # 16 chunks per batch
KTILE = 2048  # L positions per kT DMA tile
KCH = KTILE // CHUNK  # W1 chunks per kT tile
SUB = 128  # L positions per W2/W3 sub-chunk (stationary width)
NSUB = CHUNK // SUB  # 4
DV = 132  # v row: 128 data + ones col + 3 pad
VT_COLS = 16  # W3 sub-chunks per v SBUF tile
NVT = L // (SUB * VT_COLS)  # 4 v tiles per batch

MODE = "bf16"  # "bf16" | "hilo" | "f32r"

F32 = mybir.dt.float32
F32R = mybir.dt.float32r
BF16 = mybir.dt.bfloat16
ACTF = mybir.ActivationFunctionType
ALU = mybir.AluOpType

_CACHE = {}


def _split_excess_waits(nc, max_waits=1):
    """walrus in this env accepts at most one sync-wait per instruction;
    move extras onto InstNoOps placed just before (same engine, in order)."""
    for fn in nc.m.functions:
        for bb in fn.blocks:
            insts = list(bb.instructions)
            new_insts = []
            for ins in insts:
                si = ins.sync_info
                waits = list(si.on_wait) if si and si.on_wait else []
                if len(waits) > max_waits:
                    extra, keep = waits[:-max_waits], waits[-max_waits:]
                    for g0 in range(0, len(extra), max_waits):
                        pre = mybir.InstNoOp(
                            name=f"{ins.name}-waitsplit{g0}",
                            engine=ins.engine,
                            ins=[],
                            outs=[],
                            sync_info=mybir.SyncInfo(
                                on_wait=extra[g0 : g0 + max_waits], on_update=[]
                            ),
                        )
                        nc.register_instruction(pre, overwrite=True)
                        new_insts.append(pre)
                    ins.sync_info = mybir.SyncInfo(
                        on_wait=keep, on_update=list(si.on_update or [])
                    )
                new_insts.append(ins)
            if len(new_insts) != len(insts):
                bb.instructions[:] = new_insts


def build_nc(mode=MODE):
    nc = bass.Bass("TRN2")
    hilo = mode == "hilo"

    if hilo:
        kh_in = nc.dram_tensor("kh", [BPC, D, L], BF16, kind="ExternalInput")
        kl_in = nc.dram_tensor("kl", [BPC, D, L], BF16, kind="ExternalInput")
        # packed consts: cols 0:4 qwq (f32), 4:68 wkh, 68:132 wkl (bf16 pairs),
        # 132 vph|vpl interleaved as one f32 col
        cst_in = nc.dram_tensor("cst", [128, 133], F32, kind="ExternalInput")
    else:
        kT_in = nc.dram_tensor("kT", [BPC, D, L], F32R, kind="ExternalInput")
        wk_in = nc.dram_tensor("wk", [D, H], F32R, kind="ExternalInput")
        vp_in = nc.dram_tensor("vp", [H, 4], F32R, kind="ExternalInput")
    v_in = nc.dram_tensor("vv", [BPC, NVT, SUB, VT_COLS * DV], F32R, kind="ExternalInput")
    if not hilo:
        qwq_in = nc.dram_tensor("qwq", [H, BPC], F32, kind="ExternalInput")
    out_d = nc.dram_tensor("out", [1, BPC * DV], F32, kind="ExternalOutput")

    with TileContext(nc) as tc:
        with (
            tc.tile_pool(name="const", bufs=1) as cpool,
            tc.tile_pool(name="kp", bufs=8) as kpool,
            tc.tile_pool(name="vp_", bufs=2 * NVT) as vpool,
            tc.tile_pool(name="hp", bufs=4) as hpool,
            tc.tile_pool(name="wp", bufs=2) as wpool,
            tc.tile_pool(name="ob", bufs=1) as opool,
            tc.tile_pool(name="pre", bufs=2, space="PSUM") as pre_pool,
            tc.tile_pool(name="sps", bufs=2, space="PSUM") as s_pool,
            tc.tile_pool(name="ops", bufs=2, space="PSUM") as o_pool,
        ):
            # HAM warm-up on zeroed tiles: needs no DMA, so the PE clock
            # gate lifts during the Tile preamble / first transfers.
            zwarm = cpool.tile([128, 512], BF16)
            nc.gpsimd.memset(zwarm[:], 0.0)
            warm_ps = pre_pool.tile([H, CHUNK], F32, tag="pre")
            for _ in range(16):
                nc.tensor.matmul(
                    warm_ps[:, :512], zwarm[:, :128], zwarm[:], start=True, stop=True
                )

            if hilo:
                cst = cpool.tile([128, 133], F32)
                nc.sync.dma_start(cst[:], cst_in[:])
                qwq = cst[:, 0:4]
                wkh = cst[:, 4:68].bitcast(BF16)
                wkl = cst[:, 68:132].bitcast(BF16)
                vph = cst[:, 132:133].bitcast(BF16)[:, 0:1]
                vpl = cst[:, 132:133].bitcast(BF16)[:, 1:2]
            else:
                qwq = cpool.tile([H, BPC], F32)
                nc.sync.dma_start(qwq[:], qwq_in[:])
                wk = cpool.tile([D, H], F32R)
                vp4 = cpool.tile([H, 4], F32R)
                nc.sync.dma_start(wk[:], wk_in[:])
                nc.sync.dma_start(vp4[:], vp_in[:])

            out_sb = opool.tile([1, BPC * DV], F32)

            def load_ktile(b, t):
                if hilo:
                    kht = kpool.tile([D, KTILE], BF16, tag="kht")
                    klt = kpool.tile([D, KTILE], BF16, tag="klt")
                    nc.sync.dma_start(kht[:], kh_in[b, :, t * CHUNK : t * CHUNK + KTILE])
                    nc.sync.dma_start(klt[:], kl_in[b, :, t * CHUNK : t * CHUNK + KTILE])
                    return (kht, klt)
                ktile = kpool.tile([D, KTILE], F32R, tag="kt")
                nc.sync.dma_start(ktile[:], kT_in[b, :, t * CHUNK : t * CHUNK + KTILE])
                return ktile

            for b in range(BPC):
                # v~ tiles for this batch (SWDGE queue so the large v
                # prefetches never head-of-line-block the kT stream, whose
                # issue rate is throttled by PE back-pressure)
                kts = {}
                if b == 0:
                    kts[0] = load_ktile(0, 0)
                    v_tiles = [None] * NVT
                else:
                    v_tiles = next_v_tiles
                next_v_tiles = [None] * NVT

                acc = o_pool.tile([1, DV], F32, tag="acc")
                w = wpool.tile([SUB, L // SUB], F32R, tag="w")
                for t in range(NCH):
                    if t % KCH == 0 and t // KCH not in kts:
                        kts[t // KCH] = load_ktile(b, t)
                    # batch 0 pulls its own v spread through its front half
                    if b == 0 and t % 2 == 0 and t // 2 < NVT:
                        vtile = vpool.tile([SUB, VT_COLS * DV], F32R, tag="vt")
                        nc.gpsimd.dma_start(vtile[:], v_in[0, t // 2])
                        v_tiles[t // 2] = vtile
                    # prefetch next batch's v in the BACK half of this batch,
                    # when the k lookahead buffers are already full
                    voff = NCH - 2 * NVT
                    if t >= voff and (t - voff) % 2 == 0 and b + 1 < BPC:
                        vt = (t - voff) // 2
                        vtile = vpool.tile([SUB, VT_COLS * DV], F32R, tag="vt")
                        nc.gpsimd.dma_start(vtile[:], v_in[b + 1, vt])
                        next_v_tiles[vt] = vtile

                    cs = slice((t % KCH) * CHUNK, (t % KCH + 1) * CHUNK)
                    pre = pre_pool.tile([H, CHUNK], F32, tag="pre")
                    if hilo:
                        kht, klt = kts[t // KCH]
                        nc.tensor.matmul(
                            pre[:], wkh[:], kht[:, cs], start=True, stop=False
                        )
                        nc.tensor.matmul(
                            pre[:], wkl[:], kht[:, cs], start=False, stop=False
                        )
                        nc.tensor.matmul(
                            pre[:], wkh[:], klt[:, cs], start=False, stop=True
                        )
                    else:
                        nc.tensor.matmul(
                            pre[:], wk[:], kts[t // KCH][:, cs], start=True, stop=True
                        )

                    if hilo:
                        h32 = hpool.tile([H, CHUNK], F32, tag="h32")
                        nc.scalar.activation(
                            h32[:], pre[:], ACTF.Tanh, bias=qwq[:, b : b + 1],
                            scale=1.0,
                        )
                        hh = hpool.tile([H, CHUNK], BF16, tag="hh")
                        nc.vector.tensor_copy(hh[:], h32[:])
                        hl = hpool.tile([H, CHUNK], BF16, tag="hl")
                        nc.vector.tensor_sub(hl[:], h32[:], hh[:])
                        if t % 4 == 0:
                            scol4 = s_pool.tile([SUB, 4 * NSUB], F32, tag="scol")
                        scol = scol4[:, (t % 4) * NSUB : (t % 4 + 1) * NSUB]
                        for j in range(NSUB):
                            js = slice(j * SUB, (j + 1) * SUB)
                            nc.tensor.matmul(
                                scol[:, j : j + 1], hh[:, js], vph[:],
                                start=True, stop=False,
                            )
                            nc.tensor.matmul(
                                scol[:, j : j + 1], hh[:, js], vpl[:],
                                start=False, stop=False,
                            )
                            nc.tensor.matmul(
                                scol[:, j : j + 1], hl[:, js], vph[:],
                                start=False, stop=True,
                            )
                        if t % 4 == 3:
                            nc.scalar.activation(
                                w[:, NSUB * (t - 3) : NSUB * (t + 1)],
                                scol4[:],
                                ACTF.Exp,
                            )
                    else:
                        hid = hpool.tile([H, CHUNK], F32R, tag="hid")
                        nc.scalar.activation(
                            hid[:], pre[:], ACTF.Tanh, bias=qwq[:, b : b + 1],
                            scale=1.0,
                        )
                        scol = s_pool.tile([SUB, 4 * NSUB], F32, tag="scol")
                        for j in range(NSUB):
                            nc.tensor.matmul(
                                scol[:, 4 * j : 4 * j + 4],
                                hid[:, j * SUB : (j + 1) * SUB],
                                vp4[:],
                                start=True,
                                stop=True,
                            )
                        nc.scalar.activation(
                            w[:, NSUB * t : NSUB * (t + 1)],
                            scol[:, 0 : 4 * NSUB : 4],
                            ACTF.Exp,
                        )

                nsub_total = L // SUB
                for tp in range(nsub_total):
                    vt, col = divmod(tp, VT_COLS)
                    nc.tensor.matmul(
                        acc[:],
                        w[:, tp : tp + 1],
                        v_tiles[vt][:, col * DV : (col + 1) * DV],
                        start=(tp == 0),
                        stop=(tp == nsub_total - 1),
                    )
                nc.scalar.copy(out_sb[:, b * DV : (b + 1) * DV], acc[:])

            nc.sync.dma_start(out_d[:], out_sb[:])

    _split_excess_waits(nc)
    return nc


def build_nc_bf16():
    """Plain-bf16 pipeline: k, hidden, w, v all bf16 (rel-err budget 2e-2
    tolerates ~1e-3 from bf16 rounding). Halves DMA bytes vs hilo/f32r and
    runs W1/W2 as single matmuls."""
    nc = bass.Bass("TRN2")
    kb_in = nc.dram_tensor("kb", [BPC, D, L], BF16, kind="ExternalInput")
    # packed consts: cols 0:4 qwq (f32), 4:68 wk (128 bf16 cols), 68 vp pair
    cst_in = nc.dram_tensor("cst", [128, 69], F32, kind="ExternalInput")
    v_in = nc.dram_tensor(
        "vv", [BPC, NVT, SUB, VT_COLS * DV], BF16, kind="ExternalInput"
    )
    out_d = nc.dram_tensor("out", [1, BPC * DV], F32, kind="ExternalOutput")

    with TileContext(nc) as tc:
        with (
            tc.tile_pool(name="const", bufs=1) as cpool,
            tc.tile_pool(name="kp", bufs=4) as kpool,
            tc.tile_pool(name="vp_", bufs=BPC * NVT) as vpool,
            tc.tile_pool(name="hp", bufs=6) as hpool,
            tc.tile_pool(name="wp", bufs=2) as wpool,
            tc.tile_pool(name="ob", bufs=1) as opool,
            tc.tile_pool(name="pre", bufs=1, space="PSUM") as pre_pool,
            tc.tile_pool(name="sps", bufs=1, space="PSUM") as s_pool,
            tc.tile_pool(name="ops", bufs=2, space="PSUM") as o_pool,
        ):
            zwarm = cpool.tile([128, 512], BF16)
            nc.gpsimd.memset(zwarm[:], 0.0)
            warm_ps = pre_pool.tile([H, CHUNK], F32, tag="pre0")
            for _ in range(16):
                nc.tensor.matmul(
                    warm_ps[:, :512], zwarm[:, :128], zwarm[:], start=True, stop=True
                )

            cst = cpool.tile([128, 69], F32)
            nc.sync.dma_start(cst[:], cst_in[:])
            qwq = cst[:, 0:4]
            wk = cst[:, 4:68].bitcast(BF16)
            vph = cst[:, 68:69].bitcast(BF16)[:, 0:1]

            out_sb = opool.tile([1, BPC * DV], F32)

            # park the LAST k tile on the SWDGE queue upfront: it sits in SBUF
            # until group 15, so the ~160GB/s HW k stream ends one tile
            # (~3.3us) earlier and the k-gated tail chain starts sooner
            NG = NCH // KCH  # ktile groups per batch (4 chunks each)
            klast = kpool.tile([D, KTILE], BF16, tag="klast", bufs=1)
            nc.gpsimd.dma_start(
                klast[:], kb_in[BPC - 1, :, (NCH - KCH) * CHUNK : NCH * CHUNK]
            )

            # all of v upfront on the SWDGE queue: 8.65 MiB fits SBUF, and the
            # last batch's W3 never waits on a just-in-time prefetch
            v_tiles = {}
            for b in range(BPC):
                for g in range(NVT):
                    vtile = vpool.tile([SUB, VT_COLS * DV], BF16, tag="vt")
                    nc.gpsimd.dma_start(vtile[:], v_in[b, g])
                    v_tiles[b, g] = vtile

            def load_ktile(b, t):
                ktile = kpool.tile([D, KTILE], BF16, tag="kt")
                nc.sync.dma_start(ktile[:], kb_in[b, :, t * CHUNK : t * CHUNK + KTILE])
                return ktile

            order = [(b, g) for b in range(BPC) for g in range(NG)]
            kts = {order[0]: load_ktile(order[0][0], 0), (BPC - 1, NG - 1): klast}
            for i, (b, g) in enumerate(order):
                if g == 0:
                    acc = o_pool.tile([1, DV], F32, tag="acc")
                    w = wpool.tile([SUB, L // SUB], BF16, tag="w")
                if True:
                    # prefetch ktiles up to 3 groups ahead (kpool bufs=4)
                    for nb, ng in order[i + 1 : i + 4]:
                        if (nb, ng) not in kts:
                            kts[nb, ng] = load_ktile(nb, ng * KCH)
                    ktile = kts.pop((b, g))
                    koff = 0

                    # 4 back-to-back W1 matmuls, same wk stationary, 4 psum banks
                    pres = []
                    for c in range(KCH):
                        pre = pre_pool.tile([H, CHUNK], F32, tag=f"pre{c}")
                        nc.tensor.matmul(
                            pre[:],
                            wk[:],
                            ktile[:, koff + c * CHUNK : koff + (c + 1) * CHUNK],
                            start=True,
                            stop=True,
                        )
                        pres.append(pre)
                    hhs = []
                    for c in range(KCH):
                        hh = hpool.tile([H, CHUNK], BF16, tag="hh")
                        nc.scalar.activation(
                            hh[:], pres[c][:], ACTF.Tanh, bias=qwq[:, b : b + 1],
                            scale=1.0,
                        )
                        hhs.append(hh)
                    # 16 score columns; stops alternate between 2 psum banks so
                    # the per-stop readout serialization overlaps
                    scolA = s_pool.tile([SUB, 2 * NSUB], F32, tag="scolA")
                    scolB = s_pool.tile([SUB, 2 * NSUB], F32, tag="scolB")
                    for c in range(KCH):
                        for j in range(NSUB):
                            sub = c * NSUB + j
                            bank = scolA if sub % 2 == 0 else scolB
                            nc.tensor.matmul(
                                bank[:, sub // 2 : sub // 2 + 1],
                                hhs[c][:, j * SUB : (j + 1) * SUB],
                                vph[:],
                                start=True,
                                stop=True,
                            )
                    # w cols [16g : 16g+8] = even subs, [16g+8 : 16g+16] = odd
                    # (v host layout is permuted to match)
                    gw = 16 * g
                    nc.scalar.activation(w[:, gw : gw + 8], scolA[:], ACTF.Exp)
                    nc.scalar.activation(w[:, gw + 8 : gw + 16], scolB[:], ACTF.Exp)

                if g == NG - 1:
                    nsub_total = L // SUB
                    for tp in range(nsub_total):
                        vt, col = divmod(tp, VT_COLS)
                        nc.tensor.matmul(
                            acc[:],
                            w[:, tp : tp + 1],
                            v_tiles[b, vt][:, col * DV : (col + 1) * DV],
                            start=(tp == 0),
                            stop=(tp == nsub_total - 1),
                        )
                    nc.scalar.copy(out_sb[:, b * DV : (b + 1) * DV], acc[:])

            nc.sync.dma_start(out_d[:], out_sb[:])

    _split_excess_waits(nc)
    return nc


def _prep_inputs(q, k, v, W_line, v_param, mode=MODE):
    """Host-side shard + layout prep. Returns per-core input maps."""
    hilo = mode == "hilo"
    bf16 = mode == "bf16"
    qWq = q.astype(np.float64) @ W_line[:D].astype(np.float64)  # (B, H)
    wk = np.ascontiguousarray(W_line[D:]).astype(np.float32)  # (D, H)

    if bf16:
        wkb = np.ascontiguousarray(wk.astype(ml_dtypes.bfloat16))
        vpb = v_param.astype(ml_dtypes.bfloat16)
        vpair = np.ascontiguousarray(
            np.stack([vpb, np.zeros_like(vpb)], axis=1)
        )  # [H, 2] bf16 -> one f32 col
    elif hilo:
        wkh = np.ascontiguousarray(wk.astype(ml_dtypes.bfloat16))
        wkl = np.ascontiguousarray(
            (wk - wkh.astype(np.float32)).astype(ml_dtypes.bfloat16)
        )
        vph = v_param.astype(ml_dtypes.bfloat16)
        vpl = (v_param - vph.astype(np.float32)).astype(ml_dtypes.bfloat16)
        vpair = np.ascontiguousarray(
            np.stack([vph, vpl], axis=1)
        )  # [H, 2] bf16 -> one f32 col
    else:
        vp4 = np.tile(v_param[:, None], (1, 4)).astype(np.float32)

    in_maps = []
    for c in range(NCORES):
        bs = slice(c * BPC, (c + 1) * BPC)
        kT = np.ascontiguousarray(k[bs].transpose(0, 2, 1))  # (BPC, D, L)
        vv = np.zeros((BPC, L, DV), dtype=np.float32)
        vv[:, :, :D] = v[bs]
        vv[:, :, D] = 1.0
        # permute into the SBUF tile layout: [b][vt][p][t*DV+d]
        vvr = vv.reshape(BPC, NVT, VT_COLS, SUB, DV)
        if bf16:
            # w cols per group come out even-subs-first (scolA) then odd (scolB)
            perm = list(range(0, VT_COLS, 2)) + list(range(1, VT_COLS, 2))
            vvr = vvr[:, :, perm]
        vv = np.ascontiguousarray(
            vvr.transpose(0, 1, 3, 2, 4).reshape(BPC, NVT, SUB, VT_COLS * DV)
        )
        qwq = np.ascontiguousarray(qWq[bs].T.astype(np.float32))  # (H, BPC)
        if bf16:
            m = {"vv": vv.astype(ml_dtypes.bfloat16)}
            kb = kT.astype(ml_dtypes.bfloat16)
            cst = np.zeros((128, 69), dtype=np.float32)
            cst[:, 0:4] = qwq
            cst[:, 4:68] = wkb.view(np.float32)
            cst[:, 68:69] = vpair.view(np.float32)
            m.update(kb=kb, cst=cst)
            in_maps.append(m)
            continue
        m = {"vv": vv}
        if hilo:
            kh = kT.astype(ml_dtypes.bfloat16)
            kl = (kT - kh.astype(np.float32)).astype(ml_dtypes.bfloat16)
            cst = np.zeros((128, 133), dtype=np.float32)
            cst[:, 0:4] = qwq
            cst[:, 4:68] = wkh.view(np.float32)
            cst[:, 68:132] = wkl.view(np.float32)
            cst[:, 132:133] = vpair.view(np.float32)
            m.update(kh=kh, kl=kl, cst=cst)
        else:
            m.update(kT=kT, wk=wk, vp=vp4, qwq=qwq)
        in_maps.append(m)
    return in_maps


def _gather_output(results):
    out = np.empty((B, D), dtype=np.float32)
    for c, r in enumerate(results):
        rows = r["out"].reshape(BPC, DV).astype(np.float64)
        out[c * BPC : (c + 1) * BPC] = (rows[:, :D] / rows[:, D : D + 1]).astype(
            np.float32
        )
    return out


def run(q, k, v, W_line, v_param, trace=False, mode=MODE, **spmd_kwargs):
    from concourse.bass_utils import run_bass_kernel_spmd

    key = ("nc", mode)
    if key not in _CACHE:
        _CACHE[key] = build_nc_bf16() if mode == "bf16" else build_nc(mode)
    nc = _CACHE[key]
    in_maps = _prep_inputs(q, k, v, W_line, v_param, mode)
    res = run_bass_kernel_spmd(
        nc, in_maps, list(range(NCORES)), trace=trace, **spmd_kwargs
    )
    return _gather_output(res.results), res


def kernel(q, k, v, W_line, v_param):
    out, _ = run(q, k, v, W_line, v_param, trace=False)
    return out



# revision 46
# speedup vs baseline: 1.0295x; 1.0295x over previous
"""Bahdanau-style additive attention on 8 TRN2 NeuronCores.

  hidden = tanh(q @ Wq + k @ Wk)        (B, L, H)
  scores = hidden @ v_param             (B, L)
  attn   = softmax(scores, axis=-1)
  out    = attn @ v                     (B, D)

Sharding: data-parallel over batch — 4 batches per core (B=32, 8 cores).

Per-core device pipeline:

  W1  preT[H, L]   = Wk.T @ kT          stationary=Wk, moving=host-transposed k
  ACT hiddenT      = tanh(preT + qWq_b) per-partition bias
  W2  scores[L, 1] = hiddenT.T @ vp     stationary=hidden chunk -> score COLUMNS
  ACT w = exp(scores)                   no max-subtraction (|scores| << 88)
  W3  acc[1, D+1]  = w.T @ [v | 1]      stationary=w column (float32r), the
                                        ones column gives the softmax
                                        denominator for free
  host: out = acc[:D] / acc[D]

MODE="bf16" (default): k, hidden, w, v all bf16 — halves DMA bytes vs
hilo/f32r (the kernel is DMA-bound) and runs W1/W2 as single matmuls;
rel err ~9e-3 against the f32 reference (gate is 2e-2). W1 runs as 4
back-to-back matmuls per 2048-col k tile (one wk stationary load, psum
banks pre0-3); W2 stops alternate two psum banks so readouts overlap;
exp runs per group as two [128,8] ACTs whose outputs land in
even/odd-interleaved w columns (v is host-permuted to match). v is
fully prefetched into SBUF upfront on the gpsimd SWDGE queue; k
streams on the sync HWDGE queue with 3-tile lookahead.

MODE="hilo": W1/W2 run as bf16 hi+lo split pairs (x = hi + lo exactly to
~2^-17), 3 matmuls each with the negligible lo*lo term dropped — fp32-grade
precision at bf16 PE speed, same DMA bytes as fp32.
MODE="f32r": W1/W2 in float32r (TF32-like, ~11-bit-mantissa RNE) — fewer
PE instructions, ~1e-3 relative error.
"""

import ml_dtypes
import numpy as np

import concourse.bass as bass
import concourse.mybir as mybir
from concourse.tile import TileContext

B, L, D, H = 32, 8192, 128, 128
NCORES = 8
BPC = B // NCORES  # batches per core
CHUNK = 512  # L positions per W1/tanh chunk (psum bank limit)
NCH = L // CHUNK  # 16 chunks per batch
KTILE = 2048  # L positions per kT DMA tile
KCH = KTILE // CHUNK  # W1 chunks per kT tile
SUB = 128  # L positions per W2/W3 sub-chunk (stationary width)
NSUB = CHUNK // SUB  # 4
DV = 132  # v row: 128 data + ones col + 3 pad
VT_COLS = 16  # W3 sub-chunks per v SBUF tile
NVT = L // (SUB * VT_COLS)  # 4 v tiles per batch

MODE = "bf16"  # "bf16" | "hilo" | "f32r"

F32 = mybir.dt.float32
F32R = mybir.dt.float32r
BF16 = mybir.dt.bfloat16
ACTF = mybir.ActivationFunctionType
ALU = mybir.AluOpType

_CACHE = {}


def _split_excess_waits(nc, max_waits=1):
    """walrus in this env accepts at most one sync-wait per instruction;
    move extras onto InstNoOps placed just before (same engine, in order)."""
    for fn in nc.m.functions:
        for bb in fn.blocks:
            insts = list(bb.instructions)
            new_insts = []
            for ins in insts:
                si = ins.sync_info
                waits = list(si.on_wait) if si and si.on_wait else []
                if len(waits) > max_waits:
                    extra, keep = waits[:-max_waits], waits[-max_waits:]
                    for g0 in range(0, len(extra), max_waits):
                        pre = mybir.InstNoOp(
                            name=f"{ins.name}-waitsplit{g0}",
                            engine=ins.engine,
                            ins=[],
                            outs=[],
                            sync_info=mybir.SyncInfo(
                                on_wait=extra[g0 : g0 + max_waits], on_update=[]
                            ),
                        )
                        nc.register_instruction(pre, overwrite=True)
                        new_insts.append(pre)
                    ins.sync_info = mybir.SyncInfo(
                        on_wait=keep, on_update=list(si.on_update or [])
                    )
                new_insts.append(ins)
            if len(new_insts) != len(insts):
                bb.instructions[:] = new_insts


def build_nc(mode=MODE):
    nc = bass.Bass("TRN2")
    hilo = mode == "hilo"

    if hilo:
        kh_in = nc.dram_tensor("kh", [BPC, D, L], BF16, kind="ExternalInput")
        kl_in = nc.dram_tensor("kl", [BPC, D, L], BF16, kind="ExternalInput")
        # packed consts: cols 0:4 qwq (f32), 4:68 wkh, 68:132 wkl (bf16 pairs),
        # 132 vph|vpl interleaved as one f32 col
        cst_in = nc.dram_tensor("cst", [128, 133], F32, kind="ExternalInput")
    else:
        kT_in = nc.dram_tensor("kT", [BPC, D, L], F32R, kind="ExternalInput")
        wk_in = nc.dram_tensor("wk", [D, H], F32R, kind="ExternalInput")
        vp_in = nc.dram_tensor("vp", [H, 4], F32R, kind="ExternalInput")
    v_in = nc.dram_tensor("vv", [BPC, NVT, SUB, VT_COLS * DV], F32R, kind="ExternalInput")
    if not hilo:
        qwq_in = nc.dram_tensor("qwq", [H, BPC], F32, kind="ExternalInput")
    out_d = nc.dram_tensor("out", [1, BPC * DV], F32, kind="ExternalOutput")

    with TileContext(nc) as tc:
        with (
            tc.tile_pool(name="const", bufs=1) as cpool,
            tc.tile_pool(name="kp", bufs=8) as kpool,
            tc.tile_pool(name="vp_", bufs=2 * NVT) as vpool,
            tc.tile_pool(name="hp", bufs=4) as hpool,
            tc.tile_pool(name="wp", bufs=2) as wpool,
            tc.tile_pool(name="ob", bufs=1) as opool,
            tc.tile_pool(name="pre", bufs=2, space="PSUM") as pre_pool,
            tc.tile_pool(name="sps", bufs=2, space="PSUM") as s_pool,
            tc.tile_pool(name="ops", bufs=2, space="PSUM") as o_pool,
        ):
            # HAM warm-up on zeroed tiles: needs no DMA, so the PE clock
            # gate lifts during the Tile preamble / first transfers.
            zwarm = cpool.tile([128, 512], BF16)
            nc.gpsimd.memset(zwarm[:], 0.0)
            warm_ps = pre_pool.tile([H, CHUNK], F32, tag="pre")
            for _ in range(16):
                nc.tensor.matmul(
                    warm_ps[:, :512], zwarm[:, :128], zwarm[:], start=True, stop=True
                )

            if hilo:
                cst = cpool.tile([128, 133], F32)
                nc.sync.dma_start(cst[:], cst_in[:])
                qwq = cst[:, 0:4]
                wkh = cst[:, 4:68].bitcast(BF16)
                wkl = cst[:, 68:132].bitcast(BF16)
                vph = cst[:, 132:133].bitcast(BF16)[:, 0:1]
                vpl = cst[:, 132:133].bitcast(BF16)[:, 1:2]
            else:
                qwq = cpool.tile([H, BPC], F32)
                nc.sync.dma_start(qwq[:], qwq_in[:])
                wk = cpool.tile([D, H], F32R)
                vp4 = cpool.tile([H, 4], F32R)
                nc.sync.dma_start(wk[:], wk_in[:])
                nc.sync.dma_start(vp4[:], vp_in[:])

            out_sb = opool.tile([1, BPC * DV], F32)

            def load_ktile(b, t):
                if hilo:
                    kht = kpool.tile([D, KTILE], BF16, tag="kht")
                    klt = kpool.tile([D, KTILE], BF16, tag="klt")
                    nc.sync.dma_start(kht[:], kh_in[b, :, t * CHUNK : t * CHUNK + KTILE])
                    nc.sync.dma_start(klt[:], kl_in[b, :, t * CHUNK : t * CHUNK + KTILE])
                    return (kht, klt)
                ktile = kpool.tile([D, KTILE], F32R, tag="kt")
                nc.sync.dma_start(ktile[:], kT_in[b, :, t * CHUNK : t * CHUNK + KTILE])
                return ktile

            for b in range(BPC):
                # v~ tiles for this batch (SWDGE queue so the large v
                # prefetches never head-of-line-block the kT stream, whose
                # issue rate is throttled by PE back-pressure)
                kts = {}
                if b == 0:
                    kts[0] = load_ktile(0, 0)
                    v_tiles = [None] * NVT
                else:
                    v_tiles = next_v_tiles
                next_v_tiles = [None] * NVT

                acc = o_pool.tile([1, DV], F32, tag="acc")
                w = wpool.tile([SUB, L // SUB], F32R, tag="w")
                for t in range(NCH):
                    if t % KCH == 0 and t // KCH not in kts:
                        kts[t // KCH] = load_ktile(b, t)
                    # batch 0 pulls its own v spread through its front half
                    if b == 0 and t % 2 == 0 and t // 2 < NVT:
                        vtile = vpool.tile([SUB, VT_COLS * DV], F32R, tag="vt")
                        nc.gpsimd.dma_start(vtile[:], v_in[0, t // 2])
                        v_tiles[t // 2] = vtile
                    # prefetch next batch's v in the BACK half of this batch,
                    # when the k lookahead buffers are already full
                    voff = NCH - 2 * NVT
                    if t >= voff and (t - voff) % 2 == 0 and b + 1 < BPC:
                        vt = (t - voff) // 2
                        vtile = vpool.tile([SUB, VT_COLS * DV], F32R, tag="vt")
                        nc.gpsimd.dma_start(vtile[:], v_in[b + 1, vt])
                        next_v_tiles[vt] = vtile

                    cs = slice((t % KCH) * CHUNK, (t % KCH + 1) * CHUNK)
                    pre = pre_pool.tile([H, CHUNK], F32, tag="pre")
                    if hilo:
                        kht, klt = kts[t // KCH]
                        nc.tensor.matmul(
                            pre[:], wkh[:], kht[:, cs], start=True, stop=False
                        )
                        nc.tensor.matmul(
                            pre[:], wkl[:], kht[:, cs], start=False, stop=False
                        )
                        nc.tensor.matmul(
                            pre[:], wkh[:], klt[:, cs], start=False, stop=True
                        )
                    else:
                        nc.tensor.matmul(
                            pre[:], wk[:], kts[t // KCH][:, cs], start=True, stop=True
                        )

                    if hilo:
                        h32 = hpool.tile([H, CHUNK], F32, tag="h32")
                        nc.scalar.activation(
                            h32[:], pre[:], ACTF.Tanh, bias=qwq[:, b : b + 1],
                            scale=1.0,
                        )
                        hh = hpool.tile([H, CHUNK], BF16, tag="hh")
                        nc.vector.tensor_copy(hh[:], h32[:])
                        hl = hpool.tile([H, CHUNK], BF16, tag="hl")
                        nc.vector.tensor_sub(hl[:], h32[:], hh[:])
                        if t % 4 == 0:
                            scol4 = s_pool.tile([SUB, 4 * NSUB], F32, tag="scol")
                        scol = scol4[:, (t % 4) * NSUB : (t % 4 + 1) * NSUB]
                        for j in range(NSUB):
                            js = slice(j * SUB, (j + 1) * SUB)
                            nc.tensor.matmul(
                                scol[:, j : j + 1], hh[:, js], vph[:],
                                start=True, stop=False,
                            )
                            nc.tensor.matmul(
                                scol[:, j : j + 1], hh[:, js], vpl[:],
                                start=False, stop=False,
                            )
                            nc.tensor.matmul(
                                scol[:, j : j + 1], hl[:, js], vph[:],
                                start=False, stop=True,
                            )
                        if t % 4 == 3:
                            nc.scalar.activation(
                                w[:, NSUB * (t - 3) : NSUB * (t + 1)],
                                scol4[:],
                                ACTF.Exp,
                            )
                    else:
                        hid = hpool.tile([H, CHUNK], F32R, tag="hid")
                        nc.scalar.activation(
                            hid[:], pre[:], ACTF.Tanh, bias=qwq[:, b : b + 1],
                            scale=1.0,
                        )
                        scol = s_pool.tile([SUB, 4 * NSUB], F32, tag="scol")
                        for j in range(NSUB):
                            nc.tensor.matmul(
                                scol[:, 4 * j : 4 * j + 4],
                                hid[:, j * SUB : (j + 1) * SUB],
                                vp4[:],
                                start=True,
                                stop=True,
                            )
                        nc.scalar.activation(
                            w[:, NSUB * t : NSUB * (t + 1)],
                            scol[:, 0 : 4 * NSUB : 4],
                            ACTF.Exp,
                        )

                nsub_total = L // SUB
                for tp in range(nsub_total):
                    vt, col = divmod(tp, VT_COLS)
                    nc.tensor.matmul(
                        acc[:],
                        w[:, tp : tp + 1],
                        v_tiles[vt][:, col * DV : (col + 1) * DV],
                        start=(tp == 0),
                        stop=(tp == nsub_total - 1),
                    )
                nc.scalar.copy(out_sb[:, b * DV : (b + 1) * DV], acc[:])

            nc.sync.dma_start(out_d[:], out_sb[:])

    _split_excess_waits(nc)
    return nc


def build_nc_bf16():
    """Plain-bf16 pipeline: k, hidden, w, v all bf16 (rel-err budget 2e-2
    tolerates ~1e-3 from bf16 rounding). Halves DMA bytes vs hilo/f32r and
    runs W1/W2 as single matmuls."""
    nc = bass.Bass("TRN2")
    kb_in = nc.dram_tensor("kb", [BPC, D, L], BF16, kind="ExternalInput")
    # packed consts: cols 0:4 qwq (f32), 4:68 wk (128 bf16 cols), 68 vp pair
    cst_in = nc.dram_tensor("cst", [128, 69], F32, kind="ExternalInput")
    v_in = nc.dram_tensor(
        "vv", [BPC, NVT, SUB, VT_COLS * DV], BF16, kind="ExternalInput"
    )
    out_d = nc.dram_tensor("out", [1, BPC * DV], F32, kind="ExternalOutput")

    with TileContext(nc) as tc:
        with (
            tc.tile_pool(name="const", bufs=1) as cpool,
            tc.tile_pool(name="kp", bufs=4) as kpool,
            tc.tile_pool(name="vp_", bufs=BPC * NVT) as vpool,
            tc.tile_pool(name="hp", bufs=6) as hpool,
            tc.tile_pool(name="wp", bufs=2) as wpool,
            tc.tile_pool(name="ob", bufs=1) as opool,
            tc.tile_pool(name="pre", bufs=1, space="PSUM") as pre_pool,
            tc.tile_pool(name="sps", bufs=1, space="PSUM") as s_pool,
            tc.tile_pool(name="ops", bufs=2, space="PSUM") as o_pool,
        ):
            zwarm = cpool.tile([128, 512], BF16)
            nc.gpsimd.memset(zwarm[:], 0.0)
            warm_ps = pre_pool.tile([H, CHUNK], F32, tag="pre0")
            for _ in range(16):
                nc.tensor.matmul(
                    warm_ps[:, :512], zwarm[:, :128], zwarm[:], start=True, stop=True
                )

            def load_ktile(b, t, split=1):
                # split>1: per-chunk DMAs (same queue/bytes) so W1 can start
                # on the first chunk before the whole tile lands — used for
                # the LAST tile, whose arrival gates the tail chain
                ktile = kpool.tile([D, KTILE], BF16, tag="kt")
                step = KTILE // split
                for s in range(split):
                    nc.sync.dma_start(
                        ktile[:, s * step : (s + 1) * step],
                        kb_in[b, :, t * CHUNK + s * step : t * CHUNK + (s + 1) * step],
                    )
                return ktile

            NG = NCH // KCH  # ktile groups per batch (4 chunks each)
            order = [(b, g) for b in range(BPC) for g in range(NG)]
            # k tile 0's descriptor goes first on the sync queue; cst follows
            # (cst is tiny and only needed by the first W1 at ~13us)
            kts = {order[0]: load_ktile(order[0][0], 0)}

            cst = cpool.tile([128, 69], F32)
            nc.sync.dma_start(cst[:], cst_in[:])
            qwq = cst[:, 0:4]
            wk = cst[:, 4:68].bitcast(BF16)
            vph = cst[:, 68:69].bitcast(BF16)[:, 0:1]

            out_sb = opool.tile([1, BPC * DV], F32)

            # all of v upfront on the SWDGE queue: 8.65 MiB fits SBUF, and the
            # last batch's W3 never waits on a just-in-time prefetch
            v_tiles = {}
            for b in range(BPC):
                for g in range(NVT):
                    vtile = vpool.tile([SUB, VT_COLS * DV], BF16, tag="vt")
                    nc.gpsimd.dma_start(vtile[:], v_in[b, g])
                    v_tiles[b, g] = vtile
            for i, (b, g) in enumerate(order):
                if g == 0:
                    acc = o_pool.tile([1, DV], F32, tag="acc")
                    w = wpool.tile([SUB, L // SUB], BF16, tag="w")
                if True:
                    # prefetch ktiles up to 3 groups ahead (kpool bufs=4)
                    for nb, ng in order[i + 1 : i + 4]:
                        if (nb, ng) not in kts:
                            kts[nb, ng] = load_ktile(
                                nb, ng * KCH, split=4 if (nb, ng) == order[-1] else 1
                            )
                    ktile = kts.pop((b, g))
                    koff = 0

                    # 4 back-to-back W1 matmuls, same wk stationary, 4 psum banks
                    pres = []
                    for c in range(KCH):
                        pre = pre_pool.tile([H, CHUNK], F32, tag=f"pre{c}")
                        nc.tensor.matmul(
                            pre[:],
                            wk[:],
                            ktile[:, koff + c * CHUNK : koff + (c + 1) * CHUNK],
                            start=True,
                            stop=True,
                        )
                        pres.append(pre)
                    hhs = []
                    for c in range(KCH):
                        hh = hpool.tile([H, CHUNK], BF16, tag="hh")
                        nc.scalar.activation(
                            hh[:], pres[c][:], ACTF.Tanh, bias=qwq[:, b : b + 1],
                            scale=1.0,
                        )
                        hhs.append(hh)
                    # 16 score columns; stops alternate between 2 psum banks so
                    # the per-stop readout serialization overlaps
                    scolA = s_pool.tile([SUB, 2 * NSUB], F32, tag="scolA")
                    scolB = s_pool.tile([SUB, 2 * NSUB], F32, tag="scolB")
                    for c in range(KCH):
                        for j in range(NSUB):
                            sub = c * NSUB + j
                            bank = scolA if sub % 2 == 0 else scolB
                            nc.tensor.matmul(
                                bank[:, sub // 2 : sub // 2 + 1],
                                hhs[c][:, j * SUB : (j + 1) * SUB],
                                vph[:],
                                start=True,
                                stop=True,
                            )
                    # w cols [16g : 16g+8] = even subs, [16g+8 : 16g+16] = odd
                    # (v host layout is permuted to match)
                    gw = 16 * g
                    nc.scalar.activation(w[:, gw : gw + 8], scolA[:], ACTF.Exp)
                    nc.scalar.activation(w[:, gw + 8 : gw + 16], scolB[:], ACTF.Exp)

                if g == NG - 1:
                    nsub_total = L // SUB
                    for tp in range(nsub_total):
                        vt, col = divmod(tp, VT_COLS)
                        nc.tensor.matmul(
                            acc[:],
                            w[:, tp : tp + 1],
                            v_tiles[b, vt][:, col * DV : (col + 1) * DV],
                            start=(tp == 0),
                            stop=(tp == nsub_total - 1),
                        )
                    nc.scalar.copy(out_sb[:, b * DV : (b + 1) * DV], acc[:])

            nc.sync.dma_start(out_d[:], out_sb[:])

    _split_excess_waits(nc)
    return nc


def _prep_inputs(q, k, v, W_line, v_param, mode=MODE):
    """Host-side shard + layout prep. Returns per-core input maps."""
    hilo = mode == "hilo"
    bf16 = mode == "bf16"
    qWq = q.astype(np.float64) @ W_line[:D].astype(np.float64)  # (B, H)
    wk = np.ascontiguousarray(W_line[D:]).astype(np.float32)  # (D, H)

    if bf16:
        wkb = np.ascontiguousarray(wk.astype(ml_dtypes.bfloat16))
        vpb = v_param.astype(ml_dtypes.bfloat16)
        vpair = np.ascontiguousarray(
            np.stack([vpb, np.zeros_like(vpb)], axis=1)
        )  # [H, 2] bf16 -> one f32 col
    elif hilo:
        wkh = np.ascontiguousarray(wk.astype(ml_dtypes.bfloat16))
        wkl = np.ascontiguousarray(
            (wk - wkh.astype(np.float32)).astype(ml_dtypes.bfloat16)
        )
        vph = v_param.astype(ml_dtypes.bfloat16)
        vpl = (v_param - vph.astype(np.float32)).astype(ml_dtypes.bfloat16)
        vpair = np.ascontiguousarray(
            np.stack([vph, vpl], axis=1)
        )  # [H, 2] bf16 -> one f32 col
    else:
        vp4 = np.tile(v_param[:, None], (1, 4)).astype(np.float32)

    in_maps = []
    for c in range(NCORES):
        bs = slice(c * BPC, (c + 1) * BPC)
        kT = np.ascontiguousarray(k[bs].transpose(0, 2, 1))  # (BPC, D, L)
        vv = np.zeros((BPC, L, DV), dtype=np.float32)
        vv[:, :, :D] = v[bs]
        vv[:, :, D] = 1.0
        # permute into the SBUF tile layout: [b][vt][p][t*DV+d]
        vvr = vv.reshape(BPC, NVT, VT_COLS, SUB, DV)
        if bf16:
            # w cols per group come out even-subs-first (scolA) then odd (scolB)
            perm = list(range(0, VT_COLS, 2)) + list(range(1, VT_COLS, 2))
            vvr = vvr[:, :, perm]
        vv = np.ascontiguousarray(
            vvr.transpose(0, 1, 3, 2, 4).reshape(BPC, NVT, SUB, VT_COLS * DV)
        )
        qwq = np.ascontiguousarray(qWq[bs].T.astype(np.float32))  # (H, BPC)
        if bf16:
            m = {"vv": vv.astype(ml_dtypes.bfloat16)}
            kb = kT.astype(ml_dtypes.bfloat16)
            cst = np.zeros((128, 69), dtype=np.float32)
            cst[:, 0:4] = qwq
            cst[:, 4:68] = wkb.view(np.float32)
            cst[:, 68:69] = vpair.view(np.float32)
            m.update(kb=kb, cst=cst)
            in_maps.append(m)
            continue
        m = {"vv": vv}
        if hilo:
            kh = kT.astype(ml_dtypes.bfloat16)
            kl = (kT - kh.astype(np.float32)).astype(ml_dtypes.bfloat16)
            cst = np.zeros((128, 133), dtype=np.float32)
            cst[:, 0:4] = qwq
            cst[:, 4:68] = wkh.view(np.float32)
            cst[:, 68:132] = wkl.view(np.float32)
            cst[:, 132:133] = vpair.view(np.float32)
            m.update(kh=kh, kl=kl, cst=cst)
        else:
            m.update(kT=kT, wk=wk, vp=vp4, qwq=qwq)
        in_maps.append(m)
    return in_maps


def _gather_output(results):
    out = np.empty((B, D), dtype=np.float32)
    for c, r in enumerate(results):
        rows = r["out"].reshape(BPC, DV).astype(np.float64)
        out[c * BPC : (c + 1) * BPC] = (rows[:, :D] / rows[:, D : D + 1]).astype(
            np.float32
        )
    return out


def run(q, k, v, W_line, v_param, trace=False, mode=MODE, **spmd_kwargs):
    from concourse.bass_utils import run_bass_kernel_spmd

    key = ("nc", mode)
    if key not in _CACHE:
        _CACHE[key] = build_nc_bf16() if mode == "bf16" else build_nc(mode)
    nc = _CACHE[key]
    in_maps = _prep_inputs(q, k, v, W_line, v_param, mode)
    res = run_bass_kernel_spmd(
        nc, in_maps, list(range(NCORES)), trace=trace, **spmd_kwargs
    )
    return _gather_output(res.results), res


def kernel(q, k, v, W_line, v_param):
    out, _ = run(q, k, v, W_line, v_param, trace=False)
    return out

